# revision 1
# baseline (speedup 1.0000x reference)
"""Trainium2 Bass kernel for BatchMatchedMSELoss.

loss = mean_i min_j mean_d (input[i,d] - target[j,d])^2

Decomposition:
  mse[i,j]  = (||x_i||^2 + ||t_j||^2 - 2<x_i, t_j>) / D
  min_j mse = (||x_i||^2 + min_j(||t_j||^2 - 2<x_i,t_j>)) / D

Sharding: each core owns a 1024-row slice of TARGET (j) and sees ALL input
rows i; j lives on PSUM partitions:
  psum[jp, i] = 2<x_i, t_j>      (fp8 e4m3 DoubleRow matmul, K=256, f32 accum)
  m'[jp, i]   = psum - tgsqc_j   (per-partition bias)
  acc[jp, i]  = max over core's j-tiles (fp16)
The bias+max-accumulate runs as one fused DVE scalar_tensor_tensor per
PSUM tile; ~2/3 of tiles instead do bias on the (otherwise idle)
Activation engine + a cheap fp16 tensor_tensor max on DVE, balancing the
engines.  Host gathers the 8 [128, 8192] fp16 partials, maxes over
(core, partition) -> rowmax'[i], rowmin = -rowmax', adds ||x_i||^2 + C,
/D, and means.

Measured on trn2 (8 cores, axon): HW exec ~79-82 us/core, loss relative
error vs the f32 reference ~1.2e-4 (fp8 cross term; exact f32 bias path).
"""

import os
import sys

sys.path.insert(0, "/opt/trn_rl_repo")

import numpy as np
import ml_dtypes

B = 8192
D = 256
NCORES = 8
JS = B // NCORES  # 1024 target rows (j) per core
P = 128
KC = D // P  # 2 contraction chunks
JT = JS // P  # 8 j-tiles per core
NT = 512  # matmul free-dim tile (one PSUM bank)
IBW = 2048  # i-block width (one PSUM tile = 4 banks)
IB = B // IBW  # 4 i-blocks
HS = IBW // NT  # 4 matmul subtiles per (j-tile, i-block)

_CACHE = {}


def _dve_path(t):
    # tile t = jt*IB + ib; ~1/3 of steady-state tiles use the fused DVE op
    return t % 3 == 0


def _build_nc():
    from contextlib import ExitStack

    import concourse.bacc as bacc
    import concourse.tile as tile
    import concourse.mybir as mybir

    bf16 = mybir.dt.bfloat16
    fp16 = mybir.dt.float16
    f32 = mybir.dt.float32

    nc = bacc.Bacc("TRN2", target_bir_lowering=False, debug=False)

    fp8 = mybir.dt.float8e4

    # tgtT: (target_shard).T (fp8 e4m3) [D, JS]; inT: (2*input).T (fp8) [D, B]
    tgtT_d = nc.dram_tensor("tgtT", [D, JS], fp8, kind="ExternalInput").ap()
    inT_d = nc.dram_tensor("inT", [D, B], fp8, kind="ExternalInput").ap()
    # negtg[p, jt] = -(||t_j||^2 - C) for local j = jt*128 + p
    negtg_d = nc.dram_tensor("negtg", [P, JT], f32, kind="ExternalInput").ap()
    # rowmax' partials over this core's j-shard; host maxes over (core, p)
    out_d = nc.dram_tensor("rowmax", [P, B], fp16, kind="ExternalOutput").ap()

    with tile.TileContext(nc) as tc, ExitStack() as ctx:
        persist = ctx.enter_context(tc.tile_pool(name="persist", bufs=1))
        psum_pool = ctx.enter_context(tc.tile_pool(name="psum", bufs=2, space="PSUM"))
        m_pool = ctx.enter_context(tc.tile_pool(name="m", bufs=6))

        # --- persistent SBUF buffers ---
        tgtT_sb = persist.tile([P, KC, JS], fp8, name="tgtT_sb", tag="tgtT_sb")
        negtg_sb = persist.tile([P, JT], f32, name="negtg_sb", tag="negtg_sb")
        inT_sb = [
            persist.tile([P, KC, IBW], fp8, name=f"inT_{ib}", tag=f"inT_{ib}")
            for ib in range(IB)
        ]
        acc = [
            persist.tile([P, IBW], fp16, name=f"acc{ib}", tag=f"acc{ib}") for ib in range(IB)
        ]

        # --- loads (weights + first i-block first, for a fast start);
        # spread across engine HWDGE queues so transfers run in parallel ---
        # first-wave "head" pieces gate MM0; tails + later i-blocks follow
        W0 = 2 * P   # first two j-tiles of weights
        I0 = NT      # first matmul subtile of inputs
        nc.sync.dma_start(out=tgtT_sb[:, 0, 0:W0], in_=tgtT_d[0:P, 0:W0])
        nc.scalar.dma_start(out=inT_sb[0][:, 0, 0:I0], in_=inT_d[0:P, 0:I0])
        nc.sync.dma_start(out=tgtT_sb[:, 1, 0:W0], in_=tgtT_d[P : 2 * P, 0:W0])
        nc.scalar.dma_start(out=inT_sb[0][:, 1, 0:I0], in_=inT_d[P : 2 * P, 0:I0])
        nc.gpsimd.dma_start(out=negtg_sb[:], in_=negtg_d[:, :])
        nc.sync.dma_start(out=tgtT_sb[:, 0, W0:JS], in_=tgtT_d[0:P, W0:JS])
        nc.scalar.dma_start(out=inT_sb[0][:, 0, I0:IBW], in_=inT_d[0:P, I0:IBW])
        nc.sync.dma_start(out=tgtT_sb[:, 1, W0:JS], in_=tgtT_d[P : 2 * P, W0:JS])
        nc.scalar.dma_start(out=inT_sb[0][:, 1, I0:IBW], in_=inT_d[P : 2 * P, I0:IBW])
        nc.sync.dma_start(out=inT_sb[1][:, 0, :], in_=inT_d[0:P, IBW : 2 * IBW])
        nc.scalar.dma_start(out=inT_sb[1][:, 1, :], in_=inT_d[P : 2 * P, IBW : 2 * IBW])
        nc.sync.dma_start(out=inT_sb[2][:, 0, :], in_=inT_d[0:P, 2 * IBW : 3 * IBW])
        nc.scalar.dma_start(out=inT_sb[2][:, 1, :], in_=inT_d[P : 2 * P, 2 * IBW : 3 * IBW])
        nc.gpsimd.dma_start(out=inT_sb[3][:, 0, :], in_=inT_d[0:P, 3 * IBW : 4 * IBW])
        nc.gpsimd.dma_start(out=inT_sb[3][:, 1, :], in_=inT_d[P : 2 * P, 3 * IBW : 4 * IBW])

        # --- main loop (jt outer: 4 independent acc chains interleave) ---
        for jt in range(JT):
            for ib in range(IB):
                t = jt * IB + ib
                ps = psum_pool.tile([P, IBW], f32)
                for h in range(HS):
                    nc.tensor.matmul(
                        ps[:, h * NT : (h + 1) * NT],
                        tgtT_sb[:, :, jt * P : (jt + 1) * P],
                        inT_sb[ib][:, :, h * NT : (h + 1) * NT],
                        start=True,
                        stop=True,
                        perf_mode=mybir.MatmulPerfMode.DoubleRow,
                    )
                bias_col = negtg_sb[:, jt : jt + 1]
                if jt == 0:
                    # initialize acc[ib] = ps + bias (fp16) via ACT
                    nc.scalar.activation(
                        out=acc[ib][:],
                        in_=ps[:],
                        func=mybir.ActivationFunctionType.Identity,
                        bias=bias_col,
                        scale=1.0,
                    )
                elif _dve_path(t):
                    # fused bias + max-accumulate on DVE
                    nc.vector.scalar_tensor_tensor(
                        out=acc[ib][:],
                        in0=ps[:],
                        scalar=bias_col,
                        in1=acc[ib][:],
                        op0=mybir.AluOpType.add,
                        op1=mybir.AluOpType.max,
                    )
                else:
                    # bias on ACT, cheap fp16 max on DVE
                    m_t = m_pool.tile([P, IBW], fp16)
                    nc.scalar.activation(
                        out=m_t[:],
                        in_=ps[:],
                        func=mybir.ActivationFunctionType.Identity,
                        bias=bias_col,
                        scale=1.0,
                    )
                    nc.vector.tensor_tensor(
                        acc[ib][:], acc[ib][:], m_t[:], op=mybir.AluOpType.max
                    )
                if jt == JT - 1:
                    out_q = (nc.sync, nc.scalar, nc.sync, nc.scalar)[ib]
                    out_q.dma_start(
                        out=out_d[:, ib * IBW : (ib + 1) * IBW], in_=acc[ib][:]
                    )

    nc.compile()
    return nc


def _get_nc():
    if "nc" not in _CACHE:
        _CACHE["nc"] = _build_nc()
    return _CACHE["nc"]


LAST_RESULTS = None  # BassKernelResults of the most recent run (for test harness)


def _install_ntff_hook_shim():
    """The image's antenv lacks axon_hooks; register an equivalent module so
    run_bass_kernel_spmd(trace=True) can capture NTFF profiles via the axon
    ctypes path.  Harmless when tracing is off."""
    import types

    try:
        import antenv.axon_hooks  # noqa: F401

        return
    except ImportError:
        pass
    hook = None
    try:
        from trn_agent_boot.trn_boot import _ntff_profile_via_ctypes

        hook = _ntff_profile_via_ctypes("/opt/axon/libaxon_pjrt.so")
    except Exception:
        pass
    import antenv

    mod = types.ModuleType("antenv.axon_hooks")
    mod.get_axon_ntff_profile_hook = lambda: hook
    mod.set_axon_ntff_profile_hook = lambda h: None
    sys.modules["antenv.axon_hooks"] = mod
    antenv.axon_hooks = mod


def kernel(input, target):
    global LAST_RESULTS
    from concourse.bass_utils import run_bass_kernel_spmd

    _install_ntff_hook_shim()

    nc = _get_nc()

    inp = np.asarray(input, dtype=np.float32)
    tgt = np.asarray(target, dtype=np.float32)
    assert inp.shape == (B, D) and tgt.shape == (B, D)

    tgtT_full = np.ascontiguousarray(tgt.T).astype(ml_dtypes.float8_e4m3)  # [D, B]
    inT_np = np.ascontiguousarray((2.0 * inp).T).astype(ml_dtypes.float8_e4m3)  # [D, B]
    tgsq = np.sum(tgt.astype(np.float64) ** 2, axis=1)
    C = float(tgsq.mean())
    tgsqc = -(tgsq - C).astype(np.float32)  # negated, centered

    in_maps = [
        {
            "tgtT": np.ascontiguousarray(tgtT_full[:, c * JS : (c + 1) * JS]),
            "inT": inT_np,
            "negtg": np.ascontiguousarray(
                tgsqc[c * JS : (c + 1) * JS].reshape(JT, P).T
            ),
        }
        for c in range(NCORES)
    ]

    trace = bool(int(os.environ.get("KERNEL_TRACE", "0")))
    res = run_bass_kernel_spmd(nc, in_maps, core_ids=list(range(NCORES)), trace=trace)
    LAST_RESULTS = res

    # rowmax'[i] = max over all cores' [128, B] partials
    partials = np.stack(
        [res.results[c]["rowmax"].astype(np.float32) for c in range(NCORES)]
    )  # [8, 128, B]
    rowmin = -partials.max(axis=(0, 1))  # [B]
    in_sq = np.sum(inp.astype(np.float64) ** 2, axis=1)
    loss = np.mean((in_sq + C + rowmin.astype(np.float64)) / float(D))
    return np.asarray(loss, dtype=np.float32)

